# revision 11
# baseline (speedup 1.0000x reference)
"""Distributed GAT (3-layer, heads=1, GlobalAttention pool) on 8 TRN2 NeuronCores.

Strategy: shard dst nodes + incident edges across 8 cores; per-layer node table
(h|1|alpha_s packed bf16 rows, 256B) is AllGathered in 4 row-quarters (each
overlapped with the transform that produces the next quarter and with edge
processing of earlier quarters); per-edge h[src] gathered by dma_gather
(4 src windows, int16 idx, 4 SWDGE queues round-robin); segment softmax via
unnormalized weights + ones-column (denominator rides the aggregation matmul;
pad slots gather a ones=0 pad row so no mask is needed); aggregation =
per-tile one-hot matmul into PSUM windows (ragged per-bin tile counts);
alpha_d[dst] expanded per edge with local_scatter + tensor_tensor_scan
(segment-hold) + PE-transpose relayout. Layer-1 table is host-precomputed.
"""

import numpy as np
import ml_dtypes

BF16 = ml_dtypes.bfloat16

# ---------------- problem constants ----------------
N = 100000
F_IN = 64
HID = 64
G = 128
L = 3
CORES = 8
NEG = 0.2

CHUNKS = 32          # dst chunks per core
NQ = 4               # src windows (pi row-quarters)
P = 128
BMAX = 32            # max tile-cols per gather batch

_cache = {}


# ================= host preprocessing =================

def _owner_bounds(n, cores):
    nl = -(-n // cores)
    return [(c * nl, min(n, (c + 1) * nl)) for c in range(cores)]


def _preprocess(edge_index):
    """Returns per-core static streams + shared schedule. Cached by id-hash."""
    key = hash(edge_index.tobytes()) if edge_index.size < (1 << 24) else hash(
        (edge_index.shape, edge_index[:, ::997].tobytes()))
    if key in _cache:
        return _cache[key]

    src_g = np.concatenate([edge_index[0], np.arange(N, dtype=np.int64)]).astype(np.int64)
    dst_g = np.concatenate([edge_index[1], np.arange(N, dtype=np.int64)]).astype(np.int64)

    bounds = _owner_bounds(N, CORES)
    nl = bounds[0][1] - bounds[0][0]
    chunk_d = -(-nl // CHUNKS)
    chunk_d = ((chunk_d + 3) // 4) * 4          # nd % 128 == 0
    nd = CHUNKS * chunk_d
    QW = nd // NQ                               # rows per core per window
    win = QW * CORES                            # rows per window
    assert win <= 32768
    assert QW % chunk_d == 0, (QW, chunk_d)     # quarters align to chunks

    owner_dst = np.minimum(dst_g // nl, CORES - 1)

    # ---- per-core relabeling pi: local dst -> [0, nd) ----
    # chunks balanced by total degree; within chunk: degree-desc order so the
    # per-column edge-count profile aligns across cores (shared-k packing).
    pis = []
    pi_inv = []
    for c in range(CORES):
        lo, hi = bounds[c]
        m = owner_dst == c
        ld = (dst_g[m] - lo).astype(np.int64)
        nl_c = hi - lo
        dq = np.zeros(nl_c, dtype=np.int64)
        np.add.at(dq, ld, 1)
        order = np.argsort(-dq, kind="stable")
        counts = np.zeros(CHUNKS, dtype=np.int64)
        loads = np.zeros(CHUNKS, dtype=np.int64)
        assign = np.full(nl_c, -1, dtype=np.int64)
        BATCH = 64
        for i0 in range(0, nl_c, BATCH):
            idxs = order[i0:i0 + BATCH]
            ch_order = np.argsort(loads + counts * 1e-6)
            k = 0
            for d in idxs:
                while counts[ch_order[k % CHUNKS]] >= chunk_d:
                    k += 1
                j = ch_order[k % CHUNKS]
                assign[d] = j
                loads[j] += dq[d]
                counts[j] += 1
                k += 1
        pi = np.full(nl_c, -1, dtype=np.int64)
        inv = np.full(nd, -1, dtype=np.int64)
        for j in range(CHUNKS):
            ds = np.where(assign == j)[0]
            ds = ds[np.argsort(-dq[ds], kind="stable")]   # degree desc
            pos = j * chunk_d + np.arange(len(ds))
            pi[ds] = pos
            inv[pos] = ds
        pis.append(pi)
        pi_inv.append(inv)

    # window-major table row of global node n:
    # row = w*win + c*QW + (pi % QW), where w = pi // QW
    trow = np.zeros(N, dtype=np.int64)
    for c in range(CORES):
        lo, hi = bounds[c]
        p = pis[c]
        w = p // QW
        trow[lo:hi] = w * win + c * QW + (p % QW)
    widx = trow % win                            # idx within window
    quad = trow[src_g] // win                    # src window

    # ---- per-core, per-bin edge lists sorted by (bin, col) ----
    core_edges = []
    for c in range(CORES):
        m = owner_dst == c
        s = src_g[m]
        d = dst_g[m] - bounds[c][0]
        q = quad[m]
        pd = pis[c][d]
        j = pd // chunk_d
        b = q * CHUNKS + j
        order = np.lexsort((pd, b))
        core_edges.append((s[order], pd[order], b[order]))

    # ---- shared k-schedule + per-core packing (joint simulation) ----
    TPBMAX = 64
    nbins = NQ * CHUNKS
    eb = [[None] * nbins for _ in range(CORES)]
    for c in range(CORES):
        s, pd, b = core_edges[c]
        for bi in range(nbins):
            m = b == bi
            eb[c][bi] = (s[m], pd[m] % chunk_d)
    ksched = np.zeros((nbins, TPBMAX), dtype=np.int64)
    ntiles = np.zeros(nbins, dtype=np.int64)
    packs = [[None] * nbins for _ in range(CORES)]
    for bi in range(nbins):
        ptr = np.zeros(CORES, dtype=np.int64)
        sizes = np.array([len(eb[c][bi][0]) for c in range(CORES)])
        tiles_per_core = [[] for _ in range(CORES)]
        tt = 0
        kprev = 0
        while np.any(ptr < sizes):
            assert tt < TPBMAX, "bin overflow"
            nxt = []
            for c in range(CORES):
                if ptr[c] < sizes[c]:
                    nxt.append(eb[c][bi][1][ptr[c]])
            k = max(kprev, min(nxt) // 64)
            ksched[bi, tt] = k
            hi_col = 64 * k + 128
            for c in range(CORES):
                s_arr, col_arr = eb[c][bi]
                take = []
                while ptr[c] < sizes[c] and len(take) < P and col_arr[ptr[c]] < hi_col:
                    take.append((s_arr[ptr[c]], col_arr[ptr[c]]))
                    ptr[c] += 1
                tiles_per_core[c].append((k, take))
            kprev = k
            tt += 1
        ntiles[bi] = max(tt, 1)
        if tt == 0:
            for c in range(CORES):
                tiles_per_core[c].append((0, []))
        for c in range(CORES):
            packs[c][bi] = tiles_per_core[c]

    tpb = int(ntiles.max())
    t_scan = tpb * P                              # rect slots per bin (A3 scan)
    toff = np.zeros(nbins + 1, dtype=np.int64)
    toff[1:] = np.cumsum(ntiles)                  # ragged tile-col offsets
    TS = int(toff[-1])                            # total tile-cols
    Sw = TS * P                                   # total ragged slots
    assert Sw % 16 == 0

    # gather batches: consecutive same-window bins, <= BMAX tile-cols
    batches = []                                  # (w, b0, b1, c0, ncols)
    bi = 0
    while bi < nbins:
        w = bi // CHUNKS
        b1 = bi
        cols = 0
        while b1 < nbins and b1 // CHUNKS == w and cols + int(ntiles[b1]) <= BMAX:
            cols += int(ntiles[b1])
            b1 += 1
        assert b1 > bi
        batches.append((w, bi, b1, int(toff[bi]), cols))
        bi = b1

    # pad slots gather a ones==0 pad row of their window
    padpos = []
    for w in range(NQ):
        found = -1
        for c in range(CORES):
            cand = np.where(pi_inv[c] == -1)[0]
            cand = cand[(cand // QW) == w]
            if len(cand):
                found = c * QW + (int(cand[0]) % QW)
                break
        assert found >= 0, f"no pad row in window {w}"
        padpos.append(found)

    # ---- emit per-core streams ----
    outs = []
    for c in range(CORES):
        idx16 = np.zeros(Sw, dtype=np.int16)
        filled = np.zeros(Sw, dtype=bool)
        colB = np.zeros((P, TS), dtype=BF16)
        keep = np.ones((P, t_scan), dtype=BF16)
        segidx = np.full((P, chunk_d), -1, dtype=np.int16)
        for bi in range(nbins):
            tiles = packs[c][bi]
            prev_col = -1
            for tt in range(int(ntiles[bi])):
                _k, take = tiles[tt] if tt < len(tiles) else (0, [])
                base = (int(toff[bi]) + tt) * P
                for sl in range(len(take)):
                    e = base + sl
                    s_g, col = take[sl]
                    idx16[e] = widx[s_g]
                    filled[e] = True
                    colB[e % P, e // P] = np.float32(col - 64 * int(ksched[bi, tt]))
                    t = tt * P + sl               # rect slot within bin
                    if col != prev_col:
                        keep[bi, t] = 0.0
                        segidx[bi, col] = t
                        prev_col = col
        for bi in range(nbins):
            w = bi // CHUNKS
            sl_ = slice(int(toff[bi]) * P, int(toff[bi + 1]) * P)
            reg = idx16[sl_]
            reg[~filled[sl_]] = padpos[w]
        iw = idx16.reshape(-1, 16).T.copy()
        idx16w = np.tile(iw, (8, 1))
        outs.append(dict(idx16w=idx16w, colB=colB, keep=keep, segidx=segidx))

    shared = dict(nd=nd, chunk_d=chunk_d, QW=QW, win=win, tpb=tpb, t_scan=t_scan,
                  ksched=ksched, ntiles=ntiles, toff=toff, TS=TS, Sw=Sw,
                  nbins=nbins, batches=batches, bounds=bounds,
                  pis=pis, pi_inv=pi_inv, trow=trow)
    _cache[key] = (outs, shared)
    return _cache[key]


# ================= device program =================

def _build_program(shared, reps=1, ablate=()):
    import concourse.bass as bass
    import concourse.mybir as mybir
    import concourse.tile as tile
    from concourse import bacc

    f32 = mybir.dt.float32
    bf16 = mybir.dt.bfloat16
    i16 = mybir.dt.int16
    AL = mybir.AluOpType
    ACTF = mybir.ActivationFunctionType

    nd = shared["nd"]; chunk_d = shared["chunk_d"]; win = shared["win"]
    QW = shared["QW"]; tpb = shared["tpb"]; t_scan = shared["t_scan"]
    nbins = shared["nbins"]; ksched = shared["ksched"]; ntiles = shared["ntiles"]
    toff = shared["toff"]; TS = shared["TS"]; Sw = shared["Sw"]
    batches = shared["batches"]
    ntab = win * NQ
    nblk = nd // P
    H1 = HID + 1
    HA = HID + 3
    adw_cols = nbins * tpb

    nc = bacc.Bacc("TRN2", target_bir_lowering=False, debug=False,
                   enable_asserts=False, num_devices=CORES, num_swdge_queues=4)

    def din(name, shape, dt):
        return nc.dram_tensor(name, shape, dt, kind="ExternalInput").ap()

    tabf0_d = din("TABF0", [ntab, P], bf16)
    ad0_d = din("AD0", [1, nd], bf16)
    idxw_d = din("idx16w", [P, Sw // 16], i16)
    colB_d = din("colB", [P, TS], bf16)
    keep_d = din("keep", [P, t_scan], bf16)
    segidx_d = din("segidx", [P, chunk_d], i16)
    colg_d = din("colg", [P, nblk], bf16)
    ident_d = din("ident", [P, P], f32)
    iota_d = din("iota128", [P, P], bf16)
    ones_d = din("ones_nd", [1, nd], bf16)
    ones2b_d = din("ones2b", [H1, HID], bf16)
    ones2f_d = din("ones2f", [H1, HID], f32)
    zeros_d = din("zeros448", [P, 448], bf16)
    waug_d = [din(f"Waug{l}", [H1, HA], bf16) for l in range(L)]
    bias_d = [din(f"bias{l}", [HID, 1], f32) for l in range(L)]
    fin_d = din("FIN", [H1, HID + 2], bf16)
    linw_d = din("LINW", [HID, HID], bf16)
    linb_d = din("LINB_REP", [G, HID], f32)
    out_d = nc.dram_tensor("out", [G, HID], f32, kind="ExternalOutput").ap()

    with tile.TileContext(nc) as tc:
        with (
            tc.tile_pool(name="dram", bufs=1, space="DRAM") as dram,
            tc.tile_pool(name="stat", bufs=1) as stat,
            tc.tile_pool(name="big", bufs=1) as big,
            tc.tile_pool(name="gbuf", bufs=3) as gpool,
            tc.tile_pool(name="bbuf", bufs=2) as bpool,
            tc.tile_pool(name="sml", bufs=2) as sml,
            tc.tile_pool(name="alay", bufs=1) as alay,
            tc.tile_pool(name="bph", bufs=2) as bph,
            tc.tile_pool(name="stg", bufs=2) as stg,
            tc.tile_pool(name="pt", bufs=2, space="PSUM") as pt,
            tc.tile_pool(name="pw", bufs=2, space="PSUM") as pw,
            tc.tile_pool(name="pp", bufs=1, space="PSUM") as ppool,
        ):
            tabs = dram.tile([nd, P], bf16)          # table shard (pi-ordered)
            tabf = dram.tile([ntab, P], bf16)        # full table, window-major
            ad_hbm = dram.tile([1, nd], bf16)
            pool_in = dram.tile([H1, G], f32)
            pool_out = dram.tile([H1, G], f32)

            # -------- statics to SBUF --------
            colB = stat.tile([P, TS], bf16)
            nc.sync.dma_start(out=colB[:], in_=colB_d[:, :])
            idxsb = stat.tile([P, Sw // 16], i16)
            nc.sync.dma_start(out=idxsb[:], in_=idxw_d[:, :])
            keep = stat.tile([P, t_scan], bf16)
            nc.sync.dma_start(out=keep[:], in_=keep_d[:, :])
            segidx = stat.tile([P, chunk_d], i16)
            nc.sync.dma_start(out=segidx[:], in_=segidx_d[:, :])
            colg = stat.tile([P, nblk], bf16)
            nc.sync.dma_start(out=colg[:], in_=colg_d[:, :])
            ident = stat.tile([P, P], f32)
            nc.sync.dma_start(out=ident[:], in_=ident_d[:, :])
            ones2b = stat.tile([H1, HID], bf16)
            nc.sync.dma_start(out=ones2b[:], in_=ones2b_d[:, :])
            ones2f = stat.tile([H1, HID], f32)
            nc.sync.dma_start(out=ones2f[:], in_=ones2f_d[:, :])
            iota128 = stat.tile([P, P], bf16)
            nc.sync.dma_start(out=iota128[:], in_=iota_d[:, :])
            zeros448 = stat.tile([P, 448], bf16)
            nc.sync.dma_start(out=zeros448[:], in_=zeros_d[:, :])
            waug = []
            for l in range(L):
                w = stat.tile([H1, HA], bf16, tag=f"waug{l}")
                nc.sync.dma_start(out=w[:], in_=waug_d[l][:, :])
                waug.append(w)
            biases = []
            for l in range(L):
                b = stat.tile([HID, 1], f32, tag=f"bias{l}")
                nc.sync.dma_start(out=b[:], in_=bias_d[l][:, :])
                biases.append(b)
            fin = stat.tile([H1, HID + 2], bf16)
            nc.sync.dma_start(out=fin[:], in_=fin_d[:, :])
            linw = stat.tile([HID, HID], bf16)
            nc.sync.dma_start(out=linw[:], in_=linw_d[:, :])
            linb = stat.tile([G, HID], f32)
            nc.sync.dma_start(out=linb[:], in_=linb_d[:, :])

            def ap_of(t, dims, offset_elems=0):
                a = t[:]
                return bass.AP(a.tensor, a.offset + offset_elems, dims)

            def body():
                x_aug = big.tile([H1, nd], bf16, tag="x_aug")
                h_aug = big.tile([HA, nd], bf16, tag="h_aug")
                out_fm = big.tile([H1, nd], bf16, tag="out_fm")
                ad_w = big.tile([P, adw_cols], f32, tag="ad_w")

                nc.sync.dma_start(out=x_aug[HID:H1, 0:nd], in_=ones_d[:, :])

                for l in range(L):
                    # ---- A1+A2: transform + table rows; quartered AllGather ----
                    if l > 0:
                        agq = 0
                        for ch0 in range(0, nd, 512):
                            cw = min(512, nd - ch0)
                            gn = cw // P
                            ps = pt.tile([P, 512], f32, tag="pt")
                            nc.tensor.matmul(
                                out=ps[0:HA, 0:cw],
                                lhsT=waug[l][:],
                                rhs=x_aug[:, ch0:ch0 + cw],
                                start=True, stop=True)
                            hch = sml.tile([HA, 512], f32, tag="hch")
                            nc.vector.tensor_copy(out=hch[:, 0:cw], in_=ps[0:HA, 0:cw])
                            nc.any.tensor_copy(out=h_aug[:, ch0:ch0 + cw], in_=ps[0:HA, 0:cw])
                            ps2 = pt.tile([P, 512], f32, tag="pt2")
                            for bb in range(gn):
                                nc.tensor.transpose(
                                    out=ps2[:, bb * HA:bb * HA + HA],
                                    in_=hch[:, bb * P:(bb + 1) * P],
                                    identity=ident[0:HA, 0:HA])
                            st = stg.tile([P, 4, P], bf16, tag="stg")
                            nc.any.tensor_copy(
                                out=ap_of(st, [st[:].ap[0], [P, gn], [1, H1]]),
                                in_=ap_of(ps2, [ps2[:].ap[0], [HA, gn], [1, H1]]))
                            stf = st[:].bitcast(f32)
                            nc.vector.tensor_copy(
                                out=bass.AP(stf.tensor, stf.offset + 33,
                                            [stf.ap[0], [P // 2, gn], [1, 1]]),
                                in_=ap_of(ps2, [ps2[:].ap[0], [HA, gn], [1, 1]],
                                          offset_elems=H1))
                            nc.sync.dma_start(
                                out=bass.AP(tabs[:].tensor,
                                            tabs[:].offset + (ch0 // P) * P * P,
                                            [[P, P], [P * P, gn], [1, P]]),
                                in_=ap_of(st, [st[:].ap[0], [P, gn], [1, P]]))
                            # emit AllGather for each completed row-quarter
                            while agq < NQ and ch0 + cw >= (agq + 1) * QW:
                                if 'collective' in ablate:
                                    nc.sync.dma_start(
                                        out=tabf[agq * win:agq * win + QW, :],
                                        in_=tabs[agq * QW:(agq + 1) * QW, :])
                                else:
                                    nc.gpsimd.collective_compute(
                                        "AllGather", AL.bypass,
                                        replica_groups=[list(range(CORES))],
                                        ins=[tabs[agq * QW:(agq + 1) * QW, :].opt()],
                                        outs=[tabf[agq * win:(agq + 1) * win, :].opt()],
                                    )
                                agq += 1
                        nc.sync.dma_start(out=ad_hbm[:, :], in_=h_aug[HID + 2:HA, 0:nd])

                    # ---- A3: alpha_d expand ----
                    a_bf = alay.tile([P, chunk_d], bf16, tag="a_bf")
                    ad_t = (ad0_d.tensor, ad0_d.offset) if l == 0 else (
                        ad_hbm[:].tensor, ad_hbm[:].offset)
                    for q in range(NQ):
                        nc.sync.dma_start(
                            out=a_bf[32 * q:32 * (q + 1), :],
                            in_=bass.AP(ad_t[0], ad_t[1],
                                        [[chunk_d, 32], [1, chunk_d]]))
                    if 'expand' in ablate:
                        nc.any.memset(ad_w[:, 0:8], 0.0)
                    if 'expand' not in ablate:
                        scat = alay.tile([P, t_scan], bf16, tag="scat")
                        nc.gpsimd.local_scatter(
                            out_ap=scat[:], data_ap=a_bf[:], idxs_ap=segidx[:],
                            channels=P, num_elems=t_scan, num_idxs=chunk_d)
                        scan = alay.tile([P, t_scan], f32, tag="scan")
                        nc.vector.tensor_tensor_scan(
                            out=scan[:], data0=keep[:], data1=scat[:], initial=0.0,
                            op0=AL.mult, op1=AL.add)
                        # relayout: ad_w[:, bin*tpb + tt] = scan[bin, tt*128 + p]
                        for b0 in range(0, tpb, 4):
                            bn = min(4, tpb - b0)
                            ps = pt.tile([P, 512], f32, tag="pt")
                            for bb in range(bn):
                                nc.tensor.transpose(
                                    out=ps[:, bb * P:(bb + 1) * P],
                                    in_=scan[:, (b0 + bb) * P:(b0 + bb + 1) * P],
                                    identity=ident[:])
                            nc.vector.tensor_copy(
                                out=ap_of(ad_w, [ad_w[:].ap[0], [tpb, P], [1, bn]],
                                          offset_elems=b0),
                                in_=ap_of(ps, [ps[:].ap[0], [1, P], [P, bn]]))

                    nc.any.memset(out_fm[:], 0.0)

                    # ---- A4: gather batches + edge compute + aggregation ----
                    for si, (w, b0, b1, c0, ncols) in enumerate(batches):
                        g = gpool.tile([P, BMAX, P], bf16, tag="g")
                        if 'gather' not in ablate:
                            nc.gpsimd.dma_gather(
                                out_ap=g[:, 0:ncols, :],
                                in_ap=(tabf0_d if l == 0 else tabf[:])[
                                    w * win:(w + 1) * win, :],
                                idxs_ap=idxsb[:, c0 * 8:(c0 + ncols) * 8],
                                num_idxs=ncols * P,
                                num_idxs_reg=ncols * P,
                                elem_size=P,
                                single_packet=False,
                                queue_num=si % 4,
                            )
                        gf = g[:].bitcast(f32)  # [P, BMAX, 64]
                        bt = bpool.tile([P, BMAX, P], bf16, tag="bt")
                        if 'edgecompute' in ablate:
                            nc.any.memset(bt[:, 0:1, 0:8], 0.0)
                        else:
                            z = sml.tile([P, BMAX], f32, tag="z")
                            for bi in range(b0, b1):
                                nt = int(ntiles[bi])
                                lo = int(toff[bi]) - c0
                                nc.vector.tensor_tensor(
                                    out=z[:, lo:lo + nt],
                                    in0=bass.AP(gf.tensor, gf.offset + lo * 64 + 33,
                                                [gf.ap[0], [P // 2, nt]]),
                                    in1=ad_w[:, bi * tpb:bi * tpb + nt], op=AL.add)
                            z5 = sml.tile([P, BMAX], f32, tag="z5")
                            nc.vector.tensor_scalar(
                                out=z5[:, 0:ncols], in0=z[:, 0:ncols],
                                scalar1=NEG, scalar2=None, op0=AL.mult)
                            nc.vector.tensor_tensor(
                                out=z[:, 0:ncols], in0=z[:, 0:ncols],
                                in1=z5[:, 0:ncols], op=AL.max)
                            peb = sml.tile([P, BMAX], bf16, tag="peb")
                            nc.scalar.activation(
                                out=peb[:, 0:ncols], in_=z[:, 0:ncols], func=ACTF.Exp)
                            # weight h (+ones) by p; pad rows have ones==0
                            nc.any.tensor_tensor(
                                out=ap_of(g, [g[:].ap[0], [P, ncols], [1, H1]]),
                                in0=ap_of(g, [g[:].ap[0], [P, ncols], [1, H1]]),
                                in1=ap_of(peb, [peb[:].ap[0], [1, ncols], [0, H1]]),
                                op=AL.mult)
                            # one-hot B
                            nc.any.tensor_tensor(
                                out=bt[:, 0:ncols, :],
                                in0=ap_of(colB, [colB[:].ap[0], [1, ncols], [0, P]],
                                          offset_elems=c0),
                                in1=ap_of(iota128, [iota128[:].ap[0], [0, ncols], [1, P]]),
                                op=AL.is_equal)
                        # aggregation matmuls per bin
                        for bi in range(b0, b1):
                            j = bi % CHUNKS
                            nt = int(ntiles[bi])
                            lo = int(toff[bi]) - c0
                            psw = pw.tile([H1, 512], f32, tag="psw")
                            nc.tensor.matmul(
                                out=psw[:, 0:448],
                                lhsT=g[:, lo, 0:H1],
                                rhs=zeros448[:],
                                start=True, stop=False, skip_group_check=True)
                            for tt in range(nt):
                                off = 64 * int(ksched[bi, tt])
                                nc.tensor.matmul(
                                    out=psw[:, off:off + P],
                                    lhsT=g[:, lo + tt, 0:H1],
                                    rhs=bt[:, lo + tt, :],
                                    start=False, stop=(tt == nt - 1),
                                    skip_group_check=True)
                            nc.any.tensor_tensor(
                                out=out_fm[:, j * chunk_d:(j + 1) * chunk_d],
                                in0=psw[:, 0:chunk_d],
                                in1=out_fm[:, j * chunk_d:(j + 1) * chunk_d],
                                op=AL.add)

                    # ---- A5: x_aug = act(num/den + b) ----
                    for ch0 in range(0, nd, 512):
                        cw = min(512, nd - ch0)
                        ps = pt.tile([P, 512], f32, tag="pt")
                        nc.tensor.matmul(
                            out=ps[0:HID, 0:cw],
                            lhsT=ones2b[HID:H1, :],
                            rhs=out_fm[HID:H1, ch0:ch0 + cw],
                            start=True, stop=True)
                        den_i = sml.tile([HID, 512], f32, tag="den_i")
                        nc.vector.tensor_scalar(
                            out=den_i[:, 0:cw], in0=ps[0:HID, 0:cw],
                            scalar1=1e-20, scalar2=None, op0=AL.add)
                        nc.vector.reciprocal(out=den_i[:, 0:cw], in_=den_i[:, 0:cw])
                        nc.vector.tensor_tensor(
                            out=x_aug[0:HID, ch0:ch0 + cw],
                            in0=out_fm[0:HID, ch0:ch0 + cw],
                            in1=den_i[:, 0:cw], op=AL.mult)
                    if l < L - 1:
                        nc.scalar.activation(
                            out=x_aug[0:HID, 0:nd], in_=x_aug[0:HID, 0:nd],
                            func=ACTF.Relu, bias=biases[l][:], scale=1.0)
                    else:
                        nc.vector.tensor_scalar(
                            out=x_aug[0:HID, 0:nd], in0=x_aug[0:HID, 0:nd],
                            scalar1=biases[l][:], scalar2=None, op0=AL.add)

                # ==================== final: pool + linear ====================
                HG = HID + 2
                x3nm = big.tile([P, nblk, HG], bf16, tag="x3share")
                for ch0 in range(0, nd, 512):
                    cw = min(512, nd - ch0)
                    gn = cw // P
                    ps = pt.tile([P, 512], f32, tag="pt")
                    nc.tensor.matmul(
                        out=ps[0:HG, 0:cw],
                        lhsT=fin[:],
                        rhs=x_aug[:, ch0:ch0 + cw],
                        start=True, stop=True)
                    hch = sml.tile([HA, 512], f32, tag="hch")
                    nc.vector.tensor_copy(out=hch[0:HG, 0:cw], in_=ps[0:HG, 0:cw])
                    ps2 = pt.tile([P, 512], f32, tag="pt2")
                    for bb in range(gn):
                        nc.tensor.transpose(
                            out=ps2[:, bb * HG:(bb + 1) * HG],
                            in_=hch[0:HG, bb * P:(bb + 1) * P],
                            identity=ident[0:HG, 0:HG])
                    nc.any.tensor_copy(
                        out=ap_of(x3nm, [x3nm[:].ap[0], [HG, gn], [1, HG]],
                                  offset_elems=(ch0 // P) * HG),
                        in_=ap_of(ps2, [ps2[:].ap[0], [HG, gn], [1, HG]]))
                wg = sml.tile([P, nblk], f32, tag="wg")
                nc.scalar.activation(
                    out=wg[:],
                    in_=ap_of(x3nm, [x3nm[:].ap[0], [HG, nblk]],
                              offset_elems=HID + 1),
                    func=ACTF.Exp)

                psp = ppool.tile([H1, G], f32, tag="pfin")
                QB = 25
                for h0 in range(0, nblk, QB):
                    hn = min(QB, nblk - h0)
                    bp = bph.tile([P, QB, G], bf16, tag="bp")
                    nc.vector.tensor_tensor(
                        out=bp[:, 0:hn, :],
                        in0=ap_of(colg, [colg[:].ap[0], [1, hn], [0, G]], offset_elems=h0),
                        in1=ap_of(iota128, [iota128[:].ap[0], [0, hn], [1, G]]),
                        op=AL.is_equal)
                    nc.vector.tensor_tensor(
                        out=bp[:, 0:hn, :], in0=bp[:, 0:hn, :],
                        in1=ap_of(wg, [wg[:].ap[0], [1, hn], [0, G]], offset_elems=h0),
                        op=AL.mult)
                    for b in range(h0, h0 + hn):
                        nc.tensor.matmul(
                            out=psp[:],
                            lhsT=x3nm[:, b, 0:H1],
                            rhs=bp[:, b - h0, :],
                            start=(b == 0), stop=(b == nblk - 1))
                pooled = sml.tile([H1, G], f32, tag="pooled")
                nc.vector.tensor_copy(out=pooled[:], in_=psp[:])
                nc.sync.dma_start(out=pool_in[:, :], in_=pooled[:])
                nc.gpsimd.collective_compute(
                    "AllReduce", AL.add,
                    replica_groups=[list(range(CORES))],
                    ins=[pool_in[:].opt()],
                    outs=[pool_out[:].opt()],
                )
                pooled_f = sml.tile([H1, G], f32, tag="pooled_f")
                nc.sync.dma_start(out=pooled_f[:], in_=pool_out[:, :])
                rec65 = sml.tile([H1, G], f32, tag="rec65")
                nc.vector.reciprocal(out=rec65[HID:H1, :], in_=pooled_f[HID:H1, :])
                psr = ppool.tile([HID, G], f32, tag="pfin")
                nc.tensor.matmul(
                    out=psr[:], lhsT=ones2f[HID:H1, :], rhs=rec65[HID:H1, :],
                    start=True, stop=True)
                pn = sml.tile([HID, G], bf16, tag="pn")
                nc.vector.tensor_tensor(
                    out=pn[:], in0=pooled_f[0:HID, :], in1=psr[:], op=AL.mult)
                psf = ppool.tile([G, HID], f32, tag="pfin")
                nc.tensor.matmul(
                    out=psf[:], lhsT=pn[:], rhs=linw[:], start=True, stop=True)
                out_sb = sml.tile([G, HID], f32, tag="out_sb")
                nc.vector.tensor_tensor(out=out_sb[:], in0=psf[:], in1=linb[:], op=AL.add)
                nc.sync.dma_start(out=out_d[:, :], in_=out_sb[:])

            for _rep in range(reps):
                body()

    nc.compile()
    return nc


# ================= entry point =================

def _prepare(x, edge_index, batch, W0, asrc0, adst0, b0, W1, asrc1, adst1, b1,
             W2, asrc2, adst2, b2, gate_w, gate_b, lin_w, lin_b, reps=1, ablate=()):
    """Build the compiled Bass program + per-core input maps."""
    x = np.asarray(x, dtype=np.float32)
    edge_index = np.asarray(edge_index)
    batch = np.asarray(batch).astype(np.int64)

    outs, shared = _preprocess(edge_index)
    nd = shared["nd"]
    QW = shared["QW"]
    win = shared["win"]
    bounds = shared["bounds"]
    nblk = nd // P

    nc = _build_program(shared, reps=reps, ablate=ablate)

    Ws = [np.asarray(W0, np.float32), np.asarray(W1, np.float32), np.asarray(W2, np.float32)]
    asr = [np.asarray(asrc0, np.float32), np.asarray(asrc1, np.float32), np.asarray(asrc2, np.float32)]
    ads = [np.asarray(adst0, np.float32), np.asarray(adst1, np.float32), np.asarray(adst2, np.float32)]
    bs = [np.asarray(b0, np.float32), np.asarray(b1, np.float32), np.asarray(b2, np.float32)]
    H1 = HID + 1
    HA = HID + 3
    waugs = []
    for l in range(L):
        w = np.zeros((H1, HA), np.float32)
        w[0:HID, 0:HID] = Ws[l]
        w[HID, HID] = 1.0
        w[0:HID, HID + 1] = Ws[l] @ asr[l]
        w[0:HID, HID + 2] = Ws[l] @ ads[l]
        waugs.append(w)
    fin = np.zeros((H1, HID + 2), np.float32)
    fin[0:HID, 0:HID] = np.eye(HID, dtype=np.float32)
    fin[HID, HID] = 1.0
    fin[0:HID, HID + 1] = np.asarray(gate_w, np.float32)[:, 0]
    fin[HID, HID + 1] = float(np.asarray(gate_b, np.float32)[0])
    linb_rep = np.tile(np.asarray(lin_b, np.float32)[None, :], (G, 1))

    ident = np.eye(P, dtype=np.float32)
    iota128 = np.tile(np.arange(P, dtype=np.float32).astype(BF16)[None, :], (P, 1))
    ones2b = np.ones((HID + 1, HID), np.float32).astype(BF16)
    ones2f = np.ones((HID + 1, HID), np.float32)
    zeros448 = np.zeros((P, 448), BF16)

    # layer-0 table + alpha_d on host (window-major table rows)
    h0 = (x @ Ws[0]).astype(np.float32)
    as0 = h0 @ asr[0]
    ad0_full = h0 @ ads[0]
    ntab = win * NQ
    tabf0 = np.zeros((ntab, P), BF16)
    ad0s = []
    for c in range(CORES):
        lo, hi = bounds[c]
        pi = shared["pis"][c]
        tl = np.zeros((nd, P), BF16)
        tl[pi, 0:HID] = h0[lo:hi].astype(BF16)
        tl[pi, HID] = 1.0
        tlv = tl.view(np.float32)
        tlv[pi, 33] = as0[lo:hi]
        for w in range(NQ):
            tabf0[w * win + c * QW:w * win + (c + 1) * QW] = tl[w * QW:(w + 1) * QW]
        ad0 = np.zeros((1, nd), np.float32)
        ad0[0, pi] = ad0_full[lo:hi]
        ad0s.append(ad0.astype(BF16))

    in_maps = []
    for c in range(CORES):
        lo, hi = bounds[c]
        pi = shared["pis"][c]
        colg = np.full((P, nblk), -1.0, np.float32).astype(BF16)
        bt = batch[lo:hi]
        colg[pi % P, pi // P] = bt.astype(np.float32).astype(BF16)
        ones_c = np.ones((1, nd), np.float32)
        ones_c[0, shared["pi_inv"][c] == -1] = 0.0
        m = dict(outs[c])
        m.update(dict(
            TABF0=tabf0, AD0=ad0s[c],
            colg=colg, ident=ident, iota128=iota128,
            ones_nd=ones_c.astype(BF16), ones2b=ones2b, ones2f=ones2f,
            zeros448=zeros448,
            FIN=fin.astype(BF16), LINW=np.asarray(lin_w, np.float32).astype(BF16),
            LINB_REP=linb_rep,
        ))
        for l in range(L):
            m[f"Waug{l}"] = waugs[l].astype(BF16)
            m[f"bias{l}"] = bs[l][:, None]
        in_maps.append(m)

    return nc, in_maps


def kernel(**inputs):
    from concourse import bass_utils

    nc, in_maps = _prepare(**inputs)
    res = bass_utils.run_bass_kernel_spmd(
        nc, in_maps, core_ids=list(range(CORES)), trace=False)
    kernel._last_results = res
    return res.results[0]["out"]
